# revision 21
# baseline (speedup 1.0000x reference)
"""Trainium2 Bass kernel for causal self-attention (nn_Casualselfatt).

Reference computes (B=2, S=2048, E=1024, H=16, D=64, fp32):
    qkv = x @ W_qkv + b_qkv ; q,k,v = split(qkv)
    q = q.reshape(B, H, S, D)   # NOTE: raw reshape, no transpose.
    ...causal softmax attention per (b,h)...
    out = res @ W_proj + b_proj

The raw reshape means head h of batch b attends over the [S, D] reshape of
rows [128h, 128h+128) of q/k/v[b].  Sharding: 32 (b,h) pairs -> 4 heads of
one batch per core (core c: b=c//4, heads 4*(c%4)..+4).  Each core computes
a partial projection output; the host sums 4 partials per batch.

On-chip: scores are built transposed ([k-part, q-free]) so the softmax
denominator rides an appended ones-column through the AV matmul.  QKV runs
in bf16 (fp32 accumulate); scores run in float32r (Q/K rounded from the
fp32 psum); the post-softmax path (att weights, V, res, W_proj) is bf16.

v2: x arrives pre-transposed (host) in bf16; QKV is a single weight pass
(N=512 matmuls over all 4 heads at once).  Q/K/V sequence positions are
stored sigma-permuted within each 128-block (slot = 64*par + 8*mm + rho for
t = 16*rho + 2*mm + par), which makes every psum->SBUF distribution copy a
contiguous-run access instead of a stride-16 scatter; copies are batched
4-dim APs split between VectorE (par=0) and ScalarE (par=1).  The causal
mask is host-permuted to match and the host un-permutes output rows.
Group-0 score/exp blocks are interleaved with the V weight chunks so the
ACT engine's softmax-exp stream (the attention-phase bottleneck) starts as
early as possible.  Softmax denominators are batched into one reciprocal
per (head-pair, group); projection output is written bf16.
"""

import os

import numpy as np
import ml_dtypes

import concourse.bass as bass
import concourse.tile as tile
from concourse import bacc, mybir
import concourse.bass_utils as bass_utils

DEBUG_TAPS = os.environ.get("KERNEL_DEBUG_TAPS") == "1"

F32 = mybir.dt.float32
F32R = mybir.dt.float32r
BF16 = mybir.dt.bfloat16

B, S, E = 2, 2048, 1024
H, D = 16, 64
N_CORES = 8
HEADS_PER_CORE = 4
ROWS = 128 * HEADS_PER_CORE  # x rows per core
NM = 24                      # qkv column chunks of 128 (q:0-7, k:8-15, v:16-23)
KT = 8                       # contraction tiles over E
NG = 4                       # q groups of 512
NB = S // 128                # 16 blocks of 128 along s'


def slot_perm():
    """perm[slot] = t: original within-block position stored at `slot`."""
    p = np.zeros(128, dtype=np.int64)
    for slot in range(128):
        par, rem = divmod(slot, 64)
        mm, rho = divmod(rem, 8)
        p[slot] = 16 * rho + 2 * mm + par
    return p


def build_program(with_qkv_bias: bool, repeat: int = 1, phases: int = 7):
    nc = bacc.Bacc("TRN2", target_bir_lowering=False, debug=False,
                   num_devices=N_CORES)

    xt_in = nc.dram_tensor("xt", [128, KT, ROWS], BF16, kind="ExternalInput")
    wqkv = nc.dram_tensor("wqkv", [NM, 128, KT, 128], BF16, kind="ExternalInput")
    wproj = nc.dram_tensor("wproj", [2, 128, E], BF16, kind="ExternalInput")
    identb_in = nc.dram_tensor("identb", [128, 64], BF16, kind="ExternalInput")
    triu_in = nc.dram_tensor("triu", [128, 128], BF16, kind="ExternalInput")
    if with_qkv_bias:
        bqkv = nc.dram_tensor("bqkv", [128, NM], F32, kind="ExternalInput")
    out = nc.dram_tensor("out", [S, E], BF16, kind="ExternalOutput")
    if DEBUG_TAPS:
        dbg_qt = nc.dram_tensor("dbg_qt", [128, 2, S], F32R,
                                kind="ExternalOutput")
        dbg_kt = nc.dram_tensor("dbg_kt", [128, 2, S], F32R,
                                kind="ExternalOutput")
        dbg_vt = nc.dram_tensor("dbg_vt", [64, HEADS_PER_CORE, S], BF16,
                                kind="ExternalOutput")
        dbg_res = nc.dram_tensor("dbg_res", [2, 128, S], BF16,
                                 kind="ExternalOutput")

    with tile.TileContext(nc) as tc:
        with (
            tc.tile_pool(name="const", bufs=1) as constp,
            tc.tile_pool(name="persist", bufs=1) as persist,
        ):
            identb = constp.tile([128, 64], BF16)
            nc.sync.dma_start(identb[:], identb_in.ap())
            triu = constp.tile([128, 128], BF16)
            nc.sync.dma_start(triu[:], triu_in.ap())
            if with_qkv_bias:
                bias_sb = constp.tile([128, NM], F32)
                nc.sync.dma_start(bias_sb[:], bqkv.ap())

            wp_sb = [persist.tile([128, E], BF16, tag=f"wp{i}", name=f"wp{i}")
                     for i in range(2)]
            for i in range(2):
                nc.sync.dma_start(wp_sb[i][:], wproj.ap()[i])

            # xT[p, kt, r]: x rows (4 heads * 128) transposed, bf16 (host)
            xT = persist.tile([128, KT, ROWS], BF16, tag="xT")
            for j in range(KT):
                nc.sync.dma_start(xT[:, j, :], xt_in.ap()[:, j, :])
            # Q/K transposed: [128 (2 heads x 64 d), hp, 2048 (sigma-slot)]
            qt = persist.tile([128, 2, S], BF16, tag="qt")
            kt_ = persist.tile([128, 2, S], BF16, tag="kt")
            # V transposed: [64 d, head, 2048 (sigma-slot)]
            vt = persist.tile([64, HEADS_PER_CORE, S], BF16, tag="vt")
            # V natural per head: 16 blocks of [128, 65] (col 64 = ones),
            # k rows sigma-permuted within each block
            vnat = [persist.tile([128, NB * 65], BF16, tag=f"vn{i}", name=f"vn{i}")
                    for i in range(4)]
            # res^T per head-pair (normalized), bf16
            res = [persist.tile([128, S], BF16, tag=f"res{i}", name=f"res{i}")
                   for i in range(2)]

            with (
                tc.tile_pool(name="wch", bufs=4) as wch,
                tc.tile_pool(name="qkvps", bufs=2, space="PSUM") as qkvps,
                tc.tile_pool(name="scps", bufs=2, space="PSUM") as scps,
                tc.tile_pool(name="avps", bufs=2, space="PSUM") as avps,
                tc.tile_pool(name="expp", bufs=24) as expp,
                tc.tile_pool(name="nrm", bufs=4) as nrm,
            ):
                def qkv_chunk(m):
                    w = wch.tile([128, KT, 128], BF16)
                    nc.sync.dma_start(w[:], wqkv.ap()[m])
                    ps = qkvps.tile([128, ROWS], F32, tag="ps",
                                    name=f"qkv{m}")
                    for j in range(KT):
                        nc.tensor.matmul(ps[:], w[:, j, :], xT[:, j, :],
                                         start=(j == 0), stop=(j == KT - 1))
                    if m < 8:
                        dest, mm = qt, m
                    elif m < 16:
                        dest, mm = kt_, m - 8
                    else:
                        dest, mm = vt, m - 16
                    off = 8 * mm
                    if m < 16:
                        # r = 128*(2*hp+pos) + 8*blk + rho
                        src5 = ps[:].rearrange(
                            "p (hp pos blk rho) -> p hp pos blk rho",
                            hp=2, pos=2, rho=8)
                        dv = dest[:].rearrange("p hp (blk s) -> p hp blk s",
                                               s=128)
                        for par in range(2):
                            for pos in range(2):
                                src = src5[64 * par:64 * par + 64, :, pos]
                                dst = dv[64 * pos:64 * pos + 64, :, :,
                                         64 * par + off:64 * par + off + 8]
                                if with_qkv_bias:
                                    nc.vector.tensor_scalar_add(
                                        dst, src,
                                        bias_sb[64 * par:64 * par + 64,
                                                m:m + 1])
                                elif par == 0:
                                    nc.vector.tensor_copy(dst, src)
                                else:
                                    nc.scalar.copy(dst, src)
                    else:
                        src4 = ps[:].rearrange(
                            "p (h blk rho) -> p h blk rho",
                            h=HEADS_PER_CORE, rho=8)
                        dv = vt[:].rearrange("p h (blk s) -> p h blk s",
                                             s=128)
                        for par in range(2):
                            src = src4[64 * par:64 * par + 64]
                            dst = dv[:, :, :, 64 * par + off:64 * par + off + 8]
                            if with_qkv_bias:
                                nc.vector.tensor_scalar_add(
                                    dst, src,
                                    bias_sb[64 * par:64 * par + 64, m:m + 1])
                            elif par == 0:
                                nc.vector.tensor_copy(dst, src)
                            else:
                                nc.scalar.copy(dst, src)

                def vtrans_head(head):
                    # V blocks: transpose [64, 128] slices to k-major order
                    for jj in range(NB):
                        vp = qkvps.tile([128, 256], BF16, tag="ps",
                                        name=f"vp{head}_{jj}")
                        nc.tensor.transpose(
                            vp[:, 0:64],
                            vt[:, head, 128 * jj:128 * jj + 128],
                            identb[0:64, :])
                        nc.vector.tensor_copy(
                            vnat[head][:, 65 * jj:65 * jj + 64],
                            vp[:, 0:64])
                    nc.vector.memset(
                        vnat[head][:].rearrange(
                            "p (jj c) -> p jj c", c=65)[:, :, 64], 1.0)

                def score_block(hp, g, kb):
                    """scores + exp (+ diag mask) for one 128-k block of a
                    512-q group; returns the bf16 exp tile."""
                    q0 = 512 * g
                    ingroup = kb >= 4 * g
                    coff = 128 * (kb - 4 * g) if ingroup else 0
                    sc = scps.tile([128, 1024], F32, tag="sc")
                    ex = expp.tile([128, 1024], BF16, tag="ex")
                    for pos in range(2):
                        so = 512 * pos
                        nc.tensor.matmul(
                            sc[:, so + coff:so + 512],
                            kt_[64 * pos:64 * pos + 64, hp,
                                128 * kb:128 * kb + 128],
                            qt[64 * pos:64 * pos + 64, hp,
                               q0 + coff:q0 + 512],
                            start=True, stop=True,
                            tile_position=(64 * pos, 0))
                    if not ingroup:
                        nc.scalar.activation(
                            ex[:], sc[:],
                            mybir.ActivationFunctionType.Exp,
                            scale=0.125)
                    else:
                        sc3 = sc[:].rearrange("p (s q) -> p s q", s=2)
                        ex3 = ex[:].rearrange("p (s q) -> p s q", s=2)
                        nc.scalar.activation(
                            ex3[:, :, coff:512],
                            sc3[:, :, coff:512],
                            mybir.ActivationFunctionType.Exp,
                            scale=0.125)
                        for pos in range(2):
                            so = 512 * pos
                            nc.vector.tensor_mul(
                                ex[:, so + coff:so + coff + 128],
                                ex[:, so + coff:so + coff + 128],
                                triu[:])
                    return ex

                def av_block(hp, g, kb, ex, av, nkb):
                    ingroup = kb >= 4 * g
                    coff = 128 * (kb - 4 * g) if ingroup else 0
                    for pos in range(2):
                        so = 512 * pos
                        head = 2 * hp + pos
                        nc.tensor.matmul(
                            av[pos][:, coff:512],
                            vnat[head][:, 65 * kb:65 * kb + 65],
                            ex[:, so + coff:so + 512],
                            start=(kb == 0), stop=(kb == nkb - 1))

                def norm_g(hp, g, av):
                    q0 = 512 * g
                    # copy av out of PSUM first: releases the psum slots for
                    # the next group's AV immediately instead of after the
                    # (lane-serial, 3.3us) reciprocal chain.
                    avs = [nrm.tile([65, 512], F32, tag=f"avs{i}",
                                    name=f"avs{hp}_{g}_{i}") for i in range(2)]
                    for pos in range(2):
                        nc.vector.tensor_copy(avs[pos][:], av[pos][:])
                    # batch both denominators into one [65,512] tile at
                    # aligned partitions {0,64}: ONE lane-serial DVE
                    # reciprocal instead of two.
                    den = nrm.tile([65, 512], F32, tag="den")
                    nc.vector.memset(den[:], 1.0)
                    nc.vector.tensor_copy(den[0:1, :], avs[0][64:65, :])
                    nc.vector.tensor_copy(den[64:65, :], avs[1][64:65, :])
                    rec = nrm.tile([65, 512], F32, tag="rec")
                    nc.vector.reciprocal(rec[:], den[:])
                    for pos in range(2):
                        if pos == 0:
                            rsrc = rec[0:1, :]
                        else:
                            # keep partition_broadcast sources at base 0
                            rec2 = nrm.tile([1, 512], F32, tag="rec2")
                            nc.vector.tensor_copy(rec2[:], rec[64:65, :])
                            rsrc = rec2[:]
                        bc = nrm.tile([64, 512], F32, tag="bc")
                        nc.gpsimd.partition_broadcast(bc[:], rsrc)
                        nc.vector.tensor_mul(
                            res[hp][64 * pos:64 * pos + 64, q0:q0 + 512],
                            avs[pos][0:64, :], bc[:])

                def attn_g(hp, g):
                    av = [avps.tile([65, 512], F32, tag="av",
                                    name=f"av{hp}_{g}_{i}") for i in range(2)]
                    nkb = 4 * g + 4
                    for kb in range(nkb):
                        ex = score_block(hp, g, kb)
                        av_block(hp, g, kb, ex, av, nkb)
                    norm_g(hp, g, av)

                def attn_g_both(g, proj_between, osb):
                    # emit BOTH head-pairs' scores interleaved so the ACT exp
                    # stream stays dense; hold hp1's exp tiles and run its AV
                    # accumulation as one exp-wait-free PE burst after hp0.
                    nkb = 4 * g + 4
                    av = [avps.tile([65, 512], F32, tag="av",
                                    name=f"avA_{g}_{i}") for i in range(2)]
                    ex1 = {}
                    for kb in range(nkb):
                        ex0 = score_block(0, g, kb)
                        ex1[kb] = score_block(1, g, kb)
                        av_block(0, g, kb, ex0, av, nkb)
                    norm_g(0, g, av)
                    if proj_between is not None:
                        proj_group(proj_between, osb)
                    av = [avps.tile([65, 512], F32, tag="av",
                                    name=f"avB_{g}_{i}") for i in range(2)]
                    for kb in range(nkb):
                        av_block(1, g, kb, ex1[kb], av, nkb)
                    norm_g(1, g, av)

                def proj_group(gg, osb):
                    for blk in range(4 * gg, 4 * gg + 4):
                        o = osb.tile([128, E], BF16, name=f"o{blk}", tag="o")
                        for f in range(2):
                            pp = qkvps.tile([128, 512], F32, tag="ps",
                                            name=f"pp{blk}_{f}")
                            for hp in range(2):
                                nc.tensor.matmul(
                                    pp[:], res[hp][:, 128 * blk:128 * blk + 128],
                                    wp_sb[hp][:, 512 * f:512 * f + 512],
                                    start=(hp == 0), stop=(hp == 1))
                            nc.vector.tensor_copy(o[:, 512 * f:512 * f + 512],
                                                  pp[:])
                        nc.sync.dma_start(
                            out.ap()[128 * blk:128 * blk + 128, :], o[:])

                def av_norm_from(hp, g, exs, nkb):
                    av = [avps.tile([65, 512], F32, tag="av",
                                    name=f"av{hp}_{g}_{i}") for i in range(2)]
                    for kbb in range(nkb):
                        av_block(hp, g, kbb, exs[kbb], av, nkb)
                    norm_g(hp, g, av)

                def body():
                    if phases & 1:
                        for m in range(8, 16):   # K chunks
                            qkv_chunk(m)
                        for m in range(0, 8):    # Q chunks
                            qkv_chunk(m)
                    if phases & 2 and phases & 1:
                        # interleave group-0 score/exp blocks (gets the ACT
                        # exp stream going) with the V weight chunks (keeps
                        # PE dense while sc psum slots recycle at exp pace)
                        g0ex = {}
                        g0blocks = [(hpp, kbb) for hpp in range(2)
                                    for kbb in range(4)]
                        for i, m in enumerate(range(16, 24)):
                            hpp, kbb = g0blocks[i]
                            g0ex[(hpp, kbb)] = score_block(hpp, 0, kbb)
                            qkv_chunk(m)
                        # hoist g1-hp0 scores between the V transposes: more
                        # early exp work for ACT while PE does transposes
                        g1ex = {}
                        for head in range(HEADS_PER_CORE):
                            vtrans_head(head)
                            g1ex[2 * head] = score_block(0, 1, 2 * head)
                            g1ex[2 * head + 1] = score_block(0, 1, 2 * head + 1)
                        for hpp in range(2):
                            av_norm_from(hpp, 0,
                                         {k: g0ex[(hpp, k)] for k in range(4)},
                                         4)
                        av_norm_from(0, 1, g1ex, 8)
                        if phases & 4:
                            attn_g(1, 1)
                            proj_group(0, osb)
                        else:
                            attn_g(1, 1)
                    elif phases & 1:
                        for m in range(16, 24):
                            qkv_chunk(m)
                        for head in range(HEADS_PER_CORE):
                            vtrans_head(head)
                    for g in range(2, NG):
                        if phases & 2:
                            attn_g_both(
                                g, g - 1 if phases & 4 else None, osb)
                    if phases & 4:
                        proj_group(NG - 1, osb)
                        if not (phases & 2):
                            for g in range(NG - 1):
                                proj_group(g, osb)

                with tc.tile_pool(name="osb", bufs=3) as osb:
                    if repeat == 1:
                        body()
                    else:
                        with tc.For_i(0, repeat, 1):
                            body()
                    if DEBUG_TAPS:
                        nc.sync.dma_start(dbg_qt.ap(), qt[:])
                        nc.sync.dma_start(dbg_kt.ap(), kt_[:])
                        nc.sync.dma_start(dbg_vt.ap(), vt[:])
                        for i in range(2):
                            nc.sync.dma_start(dbg_res.ap()[i], res[i][:])

    nc.compile()
    return nc


_CACHE = {}


def _get_program(with_qkv_bias: bool):
    if with_qkv_bias not in _CACHE:
        _CACHE[with_qkv_bias] = build_program(with_qkv_bias)
    return _CACHE[with_qkv_bias]


def make_in_maps(x, W_qkv, b_qkv, W_proj):
    """Build the 8 per-core input maps (host-side data marshaling only)."""
    x = np.ascontiguousarray(np.asarray(x, dtype=np.float32))
    W_qkv = np.asarray(W_qkv, dtype=np.float32)
    b_qkv = np.asarray(b_qkv, dtype=np.float32)
    W_proj = np.asarray(W_proj, dtype=np.float32)

    wq_t = np.ascontiguousarray(
        W_qkv.astype(ml_dtypes.bfloat16).reshape(KT, 128, NM, 128)
        .transpose(2, 1, 0, 3))
    wp_b = W_proj.astype(ml_dtypes.bfloat16)
    identb = np.vstack([np.eye(64), np.eye(64)]).astype(ml_dtypes.bfloat16)
    # causal mask for diagonal blocks in sigma-slot order: visible k<=q
    perm = slot_perm()
    triu = (perm[:, None] <= perm[None, :]).astype(ml_dtypes.bfloat16)
    with_bias = bool(np.any(b_qkv))
    bias_t = np.ascontiguousarray(b_qkv.reshape(NM, 128).T) if with_bias else None

    in_maps = []
    for c in range(N_CORES):
        b, qi = c // 4, c % 4
        xc = x[b, ROWS * qi:ROWS * qi + ROWS, :]  # [512 rows, 1024]
        # xT[p, j, r] = xc[r, 128j+p], bf16
        xt = np.ascontiguousarray(
            xc.T.reshape(KT, 128, ROWS).transpose(1, 0, 2)
            .astype(ml_dtypes.bfloat16))
        m = {
            "xt": xt,
            "wqkv": wq_t,
            "wproj": np.ascontiguousarray(
                wp_b[256 * qi:256 * qi + 256, :].reshape(2, 128, E)),
            "identb": identb,
            "triu": triu,
        }
        if with_bias:
            m["bqkv"] = bias_t
        in_maps.append(m)
    return in_maps, with_bias


def kernel(x, W_qkv, b_qkv, W_proj, b_proj, _run_kwargs=None):
    in_maps, with_bias = make_in_maps(x, W_qkv, b_qkv, W_proj)
    nc = _get_program(with_bias)
    res = bass_utils.run_bass_kernel_spmd(
        nc, in_maps, core_ids=list(range(N_CORES)), **(_run_kwargs or {}))
    acc = np.zeros((B, S, E), np.float32)
    for c in range(N_CORES):
        acc[c // 4] += np.asarray(res.results[c]["out"], dtype=np.float32)
    # un-permute sequence rows (sigma-slot -> natural) within each 128-block
    perm = slot_perm()
    out = np.empty_like(acc)
    out.reshape(B, NB, 128, E)[:, :, perm, :] = acc.reshape(B, NB, 128, E)
    out += np.asarray(b_proj, dtype=np.float32)[None, None, :]
    if _run_kwargs:
        kernel.last_results = res
    return out
